# revision 22
# baseline (speedup 1.0000x reference)
"""DiffuseRouter kernel for 8 TRN2 NeuronCores.

Reference computation (enable_time=False, soft_time_routing=True):
    out[b, l, d] = (1/3) * sum_g sum_e expert_emb_g[e, b, l, d]
i.e. a uniform-weighted sum of 28 expert planes per batch element.

Sharding: pure data-parallel over batch B=8 -> one batch element per core.
Each core reads its 28 [256, 1280] f32 planes (36.7 MB), reduces them
on-chip, scales by 1/3, and writes its [256, 1280] output.  No collectives
needed (B == n_cores).

Engine assignment (v3): the DMA stream sustains ~425 GB/s aggregate
(plane completion paces at straggler SDMA engine 15, ~3.6 us/plane), so
the reduction is split across two engines that each keep pace:

  * TensorE sums free-dim columns [0, 1536) via identity matmuls
    accumulating into 3 PSUM banks (fp32r moving operand, 1 cycle/row;
    fp32r never leaves the 1.2 GHz MID clock, so a full 5-bank PE
    version at ~3.9 us/plane would throttle the stream -- 3 banks run
    at ~2.4 us/plane).  ACT applies the final x1/3 from PSUM per bank.
  * DVE sums columns [1536, 2560) with a scalar_tensor_tensor chain
    (fp32 1x mode, ~1.2 us/plane) with the 1/3 scale folded in.

Only the natural [128, 2560] full-partition contiguous plane transfer
runs the SDMA engines at line rate (~27 GB/s each); every partial or
repacked shape measured 20-30% slower (engine/port misalignment), so
engine 15's ~22 GB/s is accepted as the pacing floor.

The last plane is loaded as bank/column chunks (PE's chunks first, the
DVE chunks after, a small 256-col final chunk) so each column range's
final op -> store fires as soon as its own chunk lands, keeping the
post-stream tail to ~2-3 us.
"""

import numpy as np

import concourse.bacc as bacc
import concourse.tile as tile
from concourse import mybir
from concourse.alu_op_type import AluOpType
from concourse.bass_utils import run_bass_kernel_spmd

N_CORES = 8
E_TOTAL = 28  # 4 + 8 + 16 experts across the 3 granularity levels
L, D = 256, 1280
P = 128  # SBUF partitions
FD = (L // P) * D  # 2560 free-dim elements per partition
BW = 512  # one 2 KB PSUM bank of f32
NB_PE = 3  # banks summed on TensorE (cols 0..1536)
DVE_LO = NB_PE * BW  # 1536: start of the DVE column range
DVE_W = FD - DVE_LO  # 1024 cols summed on DVE
SCALE = 1.0 / 3.0

_NC_CACHE = None


def _build_nc():
    """Build the SPMD Bass program (identical on all 8 cores)."""
    nc = bacc.Bacc(
        "TRN2", target_bir_lowering=False, debug=False, enable_partition_id=False
    )
    f32 = mybir.dt.float32
    f32r = mybir.dt.float32r
    x = nc.dram_tensor("x", [E_TOTAL, L, D], f32, kind="ExternalInput")
    ident_d = nc.dram_tensor("ident", [P, P], f32, kind="ExternalInput")
    out = nc.dram_tensor("out", [L, D], f32, kind="ExternalOutput")

    # [E, 256, 1280] -> [E, 128, 2560]: partition p holds rows 2p, 2p+1
    # (contiguous 10240 B per partition -> fully linear 1.31 MB DMA per plane).
    x_t = x.ap().rearrange("e (p a) d -> e p (a d)", a=2)
    x_tr = x_t.bitcast(f32r)
    out_t = out.ap().rearrange("(p a) d -> p (a d)", a=2)

    mult = AluOpType.mult
    add = AluOpType.add

    with tile.TileContext(nc) as tc:
        with (
            tc.tile_pool(name="in", bufs=8) as pin,
            tc.tile_pool(name="const", bufs=1) as pconst,
            tc.tile_pool(name="acc", bufs=1) as pacc,
            tc.tile_pool(name="ps", bufs=1, space="PSUM") as pps,
        ):
            ident = pconst.tile([P, P], f32r, name="ident", tag="ident")
            # Identity comes in from DRAM on the ACT ring so the SP ring
            # carries nothing but the 28 plane loads.
            nc.scalar.dma_start(out=ident[:], in_=ident_d.ap().bitcast(f32r))
            psums = [
                pps.tile([P, BW], f32, name=f"ps{b}", tag=f"ps{b}")
                for b in range(NB_PE)
            ]
            # ACT staging for the PE banks' scaled output.
            outs = pacc.tile([P, NB_PE * BW], f32, name="outs", tag="outs")
            # DVE accumulator for cols [1536, 2560), scale folded into adds.
            acc = pacc.tile([P, DVE_W], f32, name="acc", tag="acc")

            last = E_TOTAL - 1
            for e in range(E_TOTAL):
                if e < last:
                    # One linear 1.31 MB load per plane; PE reads the f32r
                    # view, DVE reads the same bytes bitcast back to f32.
                    t = pin.tile([P, FD], f32r)
                    nc.sync.dma_start(out=t[:], in_=x_tr[e])
                    pe_chunks = [t[:, b * BW : (b + 1) * BW] for b in range(NB_PE)]
                    dve_chunks = [(DVE_LO, DVE_W, t[:, DVE_LO:FD].bitcast(f32))]
                else:
                    # Last plane: bank/column chunk loads in separate tiles so
                    # each column range's final op starts as soon as its own
                    # chunk lands.  PE chunks load first; the DVE range loads
                    # as a 768-col chunk then a small 256-col final chunk so
                    # the very last add+store is short.
                    pe_chunks = []
                    for b in range(NB_PE):
                        ct = pin.tile([P, BW], f32r, name=f"c{b}", tag=f"c{b}")
                        nc.sync.dma_start(
                            out=ct[:], in_=x_tr[e][:, b * BW : (b + 1) * BW]
                        )
                        pe_chunks.append(ct[:])
                    dve_chunks = []
                    for lo, w in ((DVE_LO, 768), (DVE_LO + 768, 256)):
                        ct = pin.tile([P, w], f32, name=f"d{lo}", tag=f"d{lo}")
                        nc.sync.dma_start(out=ct[:], in_=x_t[e][:, lo : lo + w])
                        dve_chunks.append((lo, w, ct[:]))

                for b in range(NB_PE):
                    # psum[b] (+)= chunk  via  I.T @ chunk, fp32r single-pass.
                    nc.tensor.matmul(
                        psums[b][:],
                        ident[:],
                        pe_chunks[b],
                        start=(e == 0),
                        stop=(e == last),
                    )
                    if e == last:
                        bs = slice(b * BW, (b + 1) * BW)
                        # ACT: out = psum * 1/3 (PSUM -> SBUF), then store on
                        # the ACT HWDGE ring (SP ring is busy with loads).
                        nc.scalar.mul(outs[:, bs], psums[b][:], SCALE)
                        nc.scalar.dma_start(out=out_t[:, bs], in_=outs[:, bs])

                for lo, w, th in dve_chunks:
                    qs = slice(lo - DVE_LO, lo - DVE_LO + w)
                    if e == 0:
                        # acc = t0 * 1/3 (tensor_scalar: 2x perf mode)
                        nc.vector.tensor_scalar_mul(acc[:, qs], th, SCALE)
                    else:
                        # acc = (t_e * 1/3) + acc
                        nc.vector.scalar_tensor_tensor(
                            acc[:, qs], th, SCALE, acc[:, qs], mult, add
                        )
                    if e == last:
                        nc.scalar.dma_start(
                            out=out_t[:, lo : lo + w], in_=acc[:, qs]
                        )
    nc.compile()
    return nc


def _get_nc():
    global _NC_CACHE
    if _NC_CACHE is None:
        _NC_CACHE = _build_nc()
    return _NC_CACHE


def _run(inputs, trace=False, trace_kwargs=None):
    e0 = np.asarray(inputs["expert_emb_0"], dtype=np.float32)
    e1 = np.asarray(inputs["expert_emb_1"], dtype=np.float32)
    e2 = np.asarray(inputs["expert_emb_2"], dtype=np.float32)
    B = e0.shape[1]
    assert B == N_CORES, f"expected B == {N_CORES}, got {B}"

    ident = np.eye(P, dtype=np.float32)
    in_maps = []
    for b in range(B):
        xb = np.concatenate([e0[:, b], e1[:, b], e2[:, b]], axis=0)
        in_maps.append({"x": np.ascontiguousarray(xb), "ident": ident})

    kw = {}
    if trace:
        kw["trace"] = True
        if trace_kwargs:
            kw.update(trace_kwargs)
    try:
        res = run_bass_kernel_spmd(_get_nc(), in_maps, list(range(N_CORES)), **kw)
    except Exception:
        # One retry: transient device errors (e.g. NRT unrecoverable after a
        # prior wedged run) usually clear on re-dispatch.
        res = run_bass_kernel_spmd(_get_nc(), in_maps, list(range(N_CORES)), **kw)
    out = np.stack([res.results[b]["out"] for b in range(B)], axis=0)
    return out.astype(np.float32, copy=False), res


def kernel(**inputs) -> np.ndarray:
    out, _ = _run(inputs, trace=False)
    return out
